# revision 14
# baseline (speedup 1.0000x reference)
"""CompressedLinear (int8 weight, per-row scale) on 8 Trainium2 NeuronCores.

Math: y[b,s,o] = sum_i x[b,s,i] * (w_int8[o,i] * scale[o]) + bias[o]

Strategy (tensor-parallel over out_features, per sharding hint):
  - Shard W/scale/bias rows across 8 cores (1376 rows each); x replicated.
  - Scale is applied to the matmul OUTPUT (algebraically identical), so the
    device matmuls run on the raw int8 weights in fp16 (int8 is exact in
    fp16). A single fp16 pass with host-side fp16 x bounds the output
    relative error at ~2e-4.
  - Each core computes yT[o_shard, s] = W_shard @ x^T.
  - Load scheduling is latency-driven: a dma_start costs ~0.7-1us of queue
    issue time, in-flight descriptors round-robin across the DMA engines
    (a flood starves whatever is urgent), and the gpsimd SWDGE *casting*
    path has ~6us fixed latency per block. So both operand streams are
    host-preprocessed fp16 ridden on exactly two hardware-DGE rings:
      * x blocks on the sync ring (multi-k-slice 3D-pattern blocks,
        1-k-slice head), one continuous depth-2 completion-chain across
        all chunks;
      * all weights, pre-cast and pre-blocked per PSUM o-group on the
        host, on the scalar ring the same way (the PE consumes one
        (x, w) k-slice pair per ~0.86us in a group sweep = ~296 GB/s
        joint, within two rings' bandwidth).
    The first (x, w) pair lands ~2.5us after the ~6.5us queue preamble;
    warm-up matmuls bridge the gap so the HAM clock gate is open when the
    real stream starts.
  - Per-partition affine (scale, bias) is fused into the PSUM eviction.
  - The very last PSUM group runs kt-inner per o-tile, and the final
    o-tile is split into two s-halves, so the last evictions/output DMAs
    stagger into the matmul stream instead of serializing at the end.
"""

import os
import numpy as np

import concourse.bass as bass
import concourse.tile as tile
from concourse import bacc, mybir
from concourse.bass_utils import run_bass_kernel_spmd

B = 1
S = 2048
I = 4096
O = 11008
N_CORES = 8
O_SHARD = O // N_CORES  # 1376
S_CHUNK = 512
P = 128
KB = 8    # k-slices per x block, chunks 1+
HEAD_BLOCKS = [1, 1, 2, 4, 4, 4, 4, 4, 4, 4]  # k-slices per head-stream block
GROUP_SIZES = (4, 4, 3)  # o-tiles per PSUM group


def build_bass(I_=I, O_SHARD_=O_SHARD, S_=S, S_CHUNK_=S_CHUNK):
    KT = I_ // P
    N_CHUNKS = S_ // S_CHUNK_
    OT = (O_SHARD_ + P - 1) // P

    MM_DT = mybir.dt.float16
    nc = bacc.Bacc("TRN2", target_bir_lowering=False, debug=False)

    # PSUM bank groups: 4+4+3 o-tiles so two adjacent groups fit in the
    # 8 banks and group transitions never wait on drains.
    groups = []
    g0 = 0
    for gsz in GROUP_SIZES:
        if g0 < OT:
            groups.append((g0, min(g0 + gsz, OT)))
            g0 += gsz
    g_width = [min(ge * P, O_SHARD_) - gs * P for gs, ge in groups]

    xt = nc.dram_tensor("xt", [I_, S_], mybir.dt.float16, kind="ExternalInput").ap()
    # weights pre-cast to fp16 and pre-blocked on the host, one dram
    # tensor per PSUM o-group: wg[g][p, kt*wid + o] = W^T[kt*128+p, c0+o]
    wg = [
        nc.dram_tensor(f"wg{g}", [P, KT * g_width[g]], mybir.dt.float16,
                       kind="ExternalInput").ap()
        for g in range(len(groups))
    ]
    scale = nc.dram_tensor("scale", [O_SHARD_], mybir.dt.float32, kind="ExternalInput").ap()
    bias = nc.dram_tensor("bias", [O_SHARD_], mybir.dt.float32, kind="ExternalInput").ap()
    yt = nc.dram_tensor("yt", [O_SHARD_, S_], mybir.dt.float32, kind="ExternalOutput").ap()

    full_t = O_SHARD_ // P
    rem = O_SHARD_ - full_t * P

    with tile.TileContext(nc) as tc:
        with (
            tc.tile_pool(name="wres", bufs=1) as wres_pool,
            tc.tile_pool(name="consts", bufs=1) as const_pool,
            tc.tile_pool(name="xc0", bufs=1) as x0_pool,
            tc.tile_pool(name="xcn", bufs=2 * (KT // KB)) as xn_pool,
            tc.tile_pool(name="outp", bufs=4) as out_pool,
            tc.tile_pool(name="psum", bufs=8, space="PSUM") as psum_pool,
        ):
            # PE warm-up: dependency-free matmuls on a zeroed tile keep the
            # PE busy from right after the preamble, so the HAM clock gate
            # opens (K=8/8) around when the first real matmuls flow.
            warm_sb = const_pool.tile([P, P], MM_DT)
            nc.any.memset(warm_sb[:], 0.0)
            warm_ps = psum_pool.tile([P, P], mybir.dt.float32, name="warm_ps", tag="psum")
            N_WARM = 16
            for i in range(N_WARM):
                nc.tensor.matmul(
                    warm_ps[:], warm_sb[:], warm_sb[:],
                    start=(i == 0), stop=(i == N_WARM - 1),
                )

            def chain(dd, dds, depth, reason):
                if len(dds) >= depth:
                    bass._add_dep_helper(
                        dd.ins, dds[-depth].ins, sync=True, reason=reason)
                dds.append(dd)

            # All x blocks ride the sync ring in ONE continuous depth-2
            # completion-chain: at most two descriptors in flight, so the
            # urgent chunk-0 head stays low-latency and later bulk chunks
            # never dilute the weight stream's bandwidth share.
            x_dds = []

            def emit_x_chunk0():
                blocks = []  # (kt0, kb, tile)
                kt0 = 0
                for i, kb in enumerate(HEAD_BLOCKS):
                    bt = x0_pool.tile([P, kb * S_CHUNK_], MM_DT, tag=f"x0b{i}")
                    src = xt[kt0 * P:(kt0 + kb) * P, 0:S_CHUNK_]
                    dd = nc.sync.dma_start(
                        bt[:].rearrange("p (kt s) -> p kt s", s=S_CHUNK_),
                        src.rearrange("(kt p) s -> p kt s", p=P))
                    chain(dd, x_dds, 2, "depth-2 chain: low-latency x head")
                    blocks.append((kt0, kb, bt))
                    kt0 += kb
                def rhs(kt, blocks=blocks):
                    for kt0, kb, bt in blocks:
                        if kt0 <= kt < kt0 + kb:
                            return bt[:, (kt - kt0) * S_CHUNK_:(kt - kt0 + 1) * S_CHUNK_]
                    raise KeyError(kt)
                return rhs

            def emit_x_chunk(sc):
                s0 = sc * S_CHUNK_
                blocks = []
                for b in range(KT // KB):
                    bt = xn_pool.tile([P, KB * S_CHUNK_], MM_DT, tag=f"xb{KB}")
                    src = xt[b * KB * P:(b + 1) * KB * P, s0:s0 + S_CHUNK_]
                    dd = nc.sync.dma_start(
                        bt[:].rearrange("p (kt s) -> p kt s", s=S_CHUNK_),
                        src.rearrange("(kt p) s -> p kt s", p=P))
                    chain(dd, x_dds, 2, "depth-2 chain: x bulk paced behind head")
                    blocks.append(bt)
                def rhs(kt, blocks=blocks):
                    return blocks[kt // KB][:, (kt % KB) * S_CHUNK_:(kt % KB + 1) * S_CHUNK_]
                return rhs

            rhs0 = emit_x_chunk0()

            # Weights: host-pre-cast fp16 on the scalar ring, group 0
            # first with the same small-head chaining, then groups 1/2
            # (their deadlines are one/two group sweeps out).
            w_dds = []
            w_blocks = [None] * len(groups)
            for g in range(len(groups)):
                wid = g_width[g]
                blocks = []  # (kt0, kb, tile)
                kt0 = 0
                sizes = HEAD_BLOCKS if g == 0 else [4] * (KT // 4)
                for i, kb in enumerate(sizes):
                    w_b = wres_pool.tile([P, kb * wid], MM_DT, tag=f"wg{g}_{i}")
                    dd = nc.scalar.dma_start(
                        w_b[:], wg[g][:, kt0 * wid:(kt0 + kb) * wid])
                    chain(dd, w_dds, 2, "depth-2 chain: w stream in sweep order")
                    blocks.append((kt0, kb, w_b))
                    kt0 += kb
                w_blocks[g] = blocks

            def w_slice_for(kt, g, ot_local, orows):
                wid = g_width[g]
                for kt0, kb, w_b in w_blocks[g]:
                    if kt0 <= kt < kt0 + kb:
                        base = (kt - kt0) * wid + ot_local * P
                        return w_b[:, base:base + orows]
                raise KeyError(kt)

            # per-partition scale/bias columns, scalar ring after the
            # weights (tiny; first needed at the first eviction ~35us in).
            scale_t = const_pool.tile([P, OT], mybir.dt.float32)
            bias_t = const_pool.tile([P, OT], mybir.dt.float32)
            if full_t:
                nc.scalar.dma_start(
                    scale_t[:, :full_t], scale[: full_t * P].rearrange("(t p) -> p t", p=P)
                )
                nc.scalar.dma_start(
                    bias_t[:, :full_t], bias[: full_t * P].rearrange("(t p) -> p t", p=P)
                )
            if rem:
                nc.scalar.dma_start(
                    scale_t[:rem, full_t:], scale[full_t * P:].rearrange("(t p) -> p t", p=rem)
                )
                nc.scalar.dma_start(
                    bias_t[:rem, full_t:], bias[full_t * P:].rearrange("(t p) -> p t", p=rem)
                )

            def evict(sc, ot, psum_t, s_off=0, s_len=None):
                s_len = S_CHUNK_ if s_len is None else s_len
                s0 = sc * S_CHUNK_ + s_off
                orows = min(P, O_SHARD_ - ot * P)
                out_t = out_pool.tile([P, S_CHUNK_], mybir.dt.float32)
                nc.vector.tensor_scalar(
                    out=out_t[:orows, :s_len],
                    in0=psum_t[:orows, :s_len],
                    scalar1=scale_t[:orows, ot:ot + 1],
                    scalar2=bias_t[:orows, ot:ot + 1],
                    op0=mybir.AluOpType.mult,
                    op1=mybir.AluOpType.add,
                )
                nc.sync.dma_start(
                    yt[ot * P:ot * P + orows, s0:s0 + s_len],
                    out_t[:orows, :s_len],
                )

            def emit_groups(sc, rhs, tail=False):
                # kt outer / o-tile inner: each x block's last reader comes
                # early in the group sweep, so next-chunk loads spread over
                # the whole chunk instead of bunching at its tail.
                for g, (g_start, g_end) in enumerate(groups):
                    last_group = tail and g == len(groups) - 1
                    if last_group:
                        # kt-inner per o-tile: each o-tile completes ~7us
                        # apart, so evictions/output DMAs overlap the
                        # remaining matmuls. The final o-tile is further
                        # split into two s-halves for the same reason.
                        for ot in range(g_start, g_end):
                            orows = min(P, O_SHARD_ - ot * P)
                            halves = ((0, S_CHUNK_),) if ot < g_end - 1 else (
                                (0, S_CHUNK_ // 2), (S_CHUNK_ // 2, S_CHUNK_ // 2))
                            for s_off, s_len in halves:
                                ps = psum_pool.tile(
                                    [P, s_len], mybir.dt.float32,
                                    name=f"psum_{sc}_{ot}_{s_off}", tag="psum",
                                )
                                for kt in range(KT):
                                    w_slice = w_slice_for(kt, g, ot - g_start, orows)
                                    nc.tensor.matmul(
                                        ps[:orows, :], w_slice,
                                        rhs(kt)[:, s_off:s_off + s_len],
                                        start=(kt == 0), stop=(kt == KT - 1),
                                    )
                                evict(sc, ot, ps, s_off, s_len)
                        continue
                    psums = {}
                    for ot in range(g_start, g_end):
                        psums[ot] = psum_pool.tile(
                            [P, S_CHUNK_], mybir.dt.float32,
                            name=f"psum_{sc}_{ot}", tag="psum",
                        )
                    for kt in range(KT):
                        for ot in range(g_start, g_end):
                            orows = min(P, O_SHARD_ - ot * P)
                            w_slice = w_slice_for(kt, g, ot - g_start, orows)
                            nc.tensor.matmul(
                                psums[ot][:orows, :], w_slice, rhs(kt),
                                start=(kt == 0), stop=(kt == KT - 1),
                            )
                    for ot in range(g_start, g_end):
                        evict(sc, ot, psums[ot])

            # Software-pipelined emission: loads for chunk sc+1 are emitted
            # before chunk sc's matmul groups, so in the per-queue FIFO
            # streams next-chunk loads sit ahead of this chunk's PSUM
            # drains.
            prev = rhs0
            for sc in range(N_CHUNKS):
                if sc + 1 < N_CHUNKS:
                    nxt = emit_x_chunk(sc + 1)
                else:
                    nxt = None
                emit_groups(sc, prev, tail=(sc == N_CHUNKS - 1))
                prev = nxt

    nc.compile()
    return nc


_NC_CACHE = None


def _get_nc():
    global _NC_CACHE
    if _NC_CACHE is None:
        _NC_CACHE = build_bass()
    return _NC_CACHE


def run(inputs, trace=False, trace_cores=None, tmpdir=None):
    x = np.asarray(inputs["x"])
    w = np.asarray(inputs["weight_int8"])
    scale = np.asarray(inputs["scale"], dtype=np.float32)
    bias = np.asarray(inputs["bias"], dtype=np.float32)

    x2d = np.ascontiguousarray(x.reshape(S, I).astype(np.float32, copy=False))
    xtr = np.ascontiguousarray(x2d.T.astype(np.float16))  # [I, S] fp16

    KT = I // P
    col_groups = []
    c = 0
    for gsz in GROUP_SIZES:
        c1 = min(c + gsz * P, O_SHARD)
        if c < c1:
            col_groups.append((c, c1))
        c = c1

    in_maps = []
    for cid in range(N_CORES):
        sl = slice(cid * O_SHARD, (cid + 1) * O_SHARD)
        wtr = w[sl, :].T.astype(np.float16)  # [I, O_SHARD] fp16 (int8 exact)
        m = {
            "xt": xtr,
            "scale": np.ascontiguousarray(scale[sl]),
            "bias": np.ascontiguousarray(bias[sl]),
        }
        for g, (c0, c1) in enumerate(col_groups):
            wid = c1 - c0
            # wg[p, kt*wid + o] = wtr[kt*128 + p, c0 + o]
            m[f"wg{g}"] = np.ascontiguousarray(
                wtr[:, c0:c1].reshape(KT, P, wid).transpose(1, 0, 2)
                .reshape(P, KT * wid))
        in_maps.append(m)

    nc = _get_nc()
    kwargs = {}
    if trace:
        kwargs["trace"] = True
        if trace_cores is not None:
            kwargs["trace_cores"] = trace_cores
        if tmpdir is not None:
            kwargs["tmpdir"] = tmpdir
    res = run_bass_kernel_spmd(nc, in_maps, core_ids=list(range(N_CORES)), **kwargs)

    yt_full = np.concatenate([res.results[c]["yt"] for c in range(N_CORES)], axis=0)
    out = np.ascontiguousarray(yt_full.T).reshape(B, S, O).astype(np.float32, copy=False)
    if trace:
        return out, res
    return out


def kernel(**inputs) -> np.ndarray:
    return run(inputs, trace=False)


# revision 17
# speedup vs baseline: 1.0259x; 1.0259x over previous
"""CompressedLinear (int8 weight, per-row scale) on 8 Trainium2 NeuronCores.

Math: y[b,s,o] = sum_i x[b,s,i] * (w_int8[o,i] * scale[o]) + bias[o]

Strategy (tensor-parallel over out_features, per sharding hint):
  - Shard W/scale/bias rows across 8 cores (1376 rows each); x replicated.
  - Scale is applied to the matmul OUTPUT (algebraically identical), so the
    device matmuls run on the raw int8 weights in fp16 (int8 is exact in
    fp16). A single fp16 pass with host-side fp16 x bounds the output
    relative error at ~2e-4.
  - Each core computes yT[o_shard, s] = W_shard @ x^T.
  - Load scheduling is latency-driven: a dma_start costs ~0.7-1us of queue
    issue time, in-flight descriptors round-robin across the DMA engines
    (a flood starves whatever is urgent), and the gpsimd SWDGE *casting*
    path has ~6us fixed latency per block. So both operand streams are
    host-preprocessed fp16 ridden on exactly two hardware-DGE rings:
      * x blocks on the sync ring (multi-k-slice 3D-pattern blocks,
        1-k-slice head), one continuous depth-2 completion-chain across
        all chunks;
      * all weights, pre-cast and pre-blocked per PSUM o-group on the
        host, on the scalar ring the same way (the PE consumes one
        (x, w) k-slice pair per ~0.86us in a group sweep = ~296 GB/s
        joint, within two rings' bandwidth).
    The first (x, w) pair lands ~2.5us after the ~6.5us queue preamble;
    warm-up matmuls bridge the gap so the HAM clock gate is open when the
    real stream starts.
  - Per-partition affine (scale, bias) is fused into the PSUM eviction.
  - The very last PSUM group runs kt-inner per o-tile, and the final
    o-tile is split into two s-halves, so the last evictions/output DMAs
    stagger into the matmul stream instead of serializing at the end.
"""

import os
import numpy as np

import concourse.bass as bass
import concourse.tile as tile
from concourse import bacc, mybir
from concourse.bass_utils import run_bass_kernel_spmd

B = 1
S = 2048
I = 4096
O = 11008
N_CORES = 8
O_SHARD = O // N_CORES  # 1376
S_CHUNK = 512
P = 128
KB = 8    # k-slices per x block, chunks 1+
HEAD_BLOCKS = [1, 1, 2, 4, 4, 4, 4, 4, 4, 4]  # k-slices per head-stream block
GROUP_SIZES = (4, 4, 3)  # o-tiles per PSUM group


def build_bass(I_=I, O_SHARD_=O_SHARD, S_=S, S_CHUNK_=S_CHUNK):
    KT = I_ // P
    N_CHUNKS = S_ // S_CHUNK_
    OT = (O_SHARD_ + P - 1) // P

    MM_DT = mybir.dt.float16
    nc = bacc.Bacc("TRN2", target_bir_lowering=False, debug=False)

    # PSUM bank groups: 4+4+3 o-tiles so two adjacent groups fit in the
    # 8 banks and group transitions never wait on drains.
    groups = []
    g0 = 0
    for gsz in GROUP_SIZES:
        if g0 < OT:
            groups.append((g0, min(g0 + gsz, OT)))
            g0 += gsz
    g_width = [min(ge * P, O_SHARD_) - gs * P for gs, ge in groups]

    xt = nc.dram_tensor("xt", [I_, S_], mybir.dt.float16, kind="ExternalInput").ap()
    # weights pre-cast to fp16 and pre-blocked on the host, one dram
    # tensor per PSUM o-group: wg[g][p, kt*wid + o] = W^T[kt*128+p, c0+o]
    wg = [
        nc.dram_tensor(f"wg{g}", [P, KT * g_width[g]], mybir.dt.float16,
                       kind="ExternalInput").ap()
        for g in range(len(groups))
    ]
    scale = nc.dram_tensor("scale", [O_SHARD_], mybir.dt.float32, kind="ExternalInput").ap()
    bias = nc.dram_tensor("bias", [O_SHARD_], mybir.dt.float32, kind="ExternalInput").ap()
    yt = nc.dram_tensor("yt", [O_SHARD_, S_], mybir.dt.float32, kind="ExternalOutput").ap()

    full_t = O_SHARD_ // P
    rem = O_SHARD_ - full_t * P

    with tile.TileContext(nc) as tc:
        with (
            tc.tile_pool(name="wres", bufs=1) as wres_pool,
            tc.tile_pool(name="consts", bufs=1) as const_pool,
            tc.tile_pool(name="xc0", bufs=1) as x0_pool,
            tc.tile_pool(name="xcn", bufs=2 * (KT // KB)) as xn_pool,
            tc.tile_pool(name="outp", bufs=4) as out_pool,
            tc.tile_pool(name="psum", bufs=8, space="PSUM") as psum_pool,
        ):
            # PE warm-up: dependency-free matmuls on a zeroed tile keep the
            # PE busy from right after the preamble, so the HAM clock gate
            # opens (K=8/8) around when the first real matmuls flow.
            warm_sb = const_pool.tile([P, P], MM_DT)
            nc.any.memset(warm_sb[:], 0.0)
            warm_ps = psum_pool.tile([P, P], mybir.dt.float32, name="warm_ps", tag="psum")
            N_WARM = 16
            for i in range(N_WARM):
                nc.tensor.matmul(
                    warm_ps[:], warm_sb[:], warm_sb[:],
                    start=(i == 0), stop=(i == N_WARM - 1),
                )

            def chain(dd, dds, reason, head_depth=2, depth=4, head_n=4):
                # depth-2 for the first head_n blocks (low first-block
                # latency), then depth-4 (more in flight = bigger share of
                # the DMA engine pool = higher stream rate).
                d = head_depth if len(dds) < head_n else depth
                if len(dds) >= d:
                    bass._add_dep_helper(
                        dd.ins, dds[-d].ins, sync=True, reason=reason)
                dds.append(dd)

            # All x blocks ride the sync ring in ONE continuous depth-2
            # completion-chain: at most two descriptors in flight, so the
            # urgent chunk-0 head stays low-latency and later bulk chunks
            # never dilute the weight stream's bandwidth share.
            x_dds = []

            def emit_x_chunk0():
                blocks = []  # (kt0, kb, tile)
                kt0 = 0
                for i, kb in enumerate(HEAD_BLOCKS):
                    bt = x0_pool.tile([P, kb * S_CHUNK_], MM_DT, tag=f"x0b{i}")
                    src = xt[kt0 * P:(kt0 + kb) * P, 0:S_CHUNK_]
                    dd = nc.sync.dma_start(
                        bt[:].rearrange("p (kt s) -> p kt s", s=S_CHUNK_),
                        src.rearrange("(kt p) s -> p kt s", p=P))
                    chain(dd, x_dds, "x stream: low-latency head, then depth-4")
                    blocks.append((kt0, kb, bt))
                    kt0 += kb
                def rhs(kt, blocks=blocks):
                    for kt0, kb, bt in blocks:
                        if kt0 <= kt < kt0 + kb:
                            return bt[:, (kt - kt0) * S_CHUNK_:(kt - kt0 + 1) * S_CHUNK_]
                    raise KeyError(kt)
                return rhs

            def emit_x_chunk(sc):
                s0 = sc * S_CHUNK_
                blocks = []
                for b in range(KT // KB):
                    bt = xn_pool.tile([P, KB * S_CHUNK_], MM_DT, tag=f"xb{KB}")
                    src = xt[b * KB * P:(b + 1) * KB * P, s0:s0 + S_CHUNK_]
                    dd = nc.sync.dma_start(
                        bt[:].rearrange("p (kt s) -> p kt s", s=S_CHUNK_),
                        src.rearrange("(kt p) s -> p kt s", p=P))
                    chain(dd, x_dds, "x stream: bulk paced behind head")
                    blocks.append(bt)
                def rhs(kt, blocks=blocks):
                    return blocks[kt // KB][:, (kt % KB) * S_CHUNK_:(kt % KB + 1) * S_CHUNK_]
                return rhs

            rhs0 = emit_x_chunk0()

            # Weights: host-pre-cast fp16 on the scalar ring, group 0
            # first with the same small-head chaining, then groups 1/2
            # (their deadlines are one/two group sweeps out).
            w_dds = []
            w_blocks = [None] * len(groups)
            for g in range(len(groups)):
                wid = g_width[g]
                blocks = []  # (kt0, kb, tile)
                kt0 = 0
                sizes = HEAD_BLOCKS if g == 0 else [4] * (KT // 4)
                for i, kb in enumerate(sizes):
                    w_b = wres_pool.tile([P, kb * wid], MM_DT, tag=f"wg{g}_{i}")
                    dd = nc.scalar.dma_start(
                        w_b[:], wg[g][:, kt0 * wid:(kt0 + kb) * wid])
                    chain(dd, w_dds, "w stream: low-latency head, then depth-4")
                    blocks.append((kt0, kb, w_b))
                    kt0 += kb
                w_blocks[g] = blocks

            def w_slice_for(kt, g, ot_local, orows):
                wid = g_width[g]
                for kt0, kb, w_b in w_blocks[g]:
                    if kt0 <= kt < kt0 + kb:
                        base = (kt - kt0) * wid + ot_local * P
                        return w_b[:, base:base + orows]
                raise KeyError(kt)

            # per-partition scale/bias columns, scalar ring after the
            # weights (tiny; first needed at the first eviction ~35us in).
            scale_t = const_pool.tile([P, OT], mybir.dt.float32)
            bias_t = const_pool.tile([P, OT], mybir.dt.float32)
            if full_t:
                nc.gpsimd.dma_start(
                    scale_t[:, :full_t], scale[: full_t * P].rearrange("(t p) -> p t", p=P)
                )
                nc.gpsimd.dma_start(
                    bias_t[:, :full_t], bias[: full_t * P].rearrange("(t p) -> p t", p=P)
                )
            if rem:
                nc.gpsimd.dma_start(
                    scale_t[:rem, full_t:], scale[full_t * P:].rearrange("(t p) -> p t", p=rem)
                )
                nc.gpsimd.dma_start(
                    bias_t[:rem, full_t:], bias[full_t * P:].rearrange("(t p) -> p t", p=rem)
                )

            def evict(sc, ot, psum_t, s_off=0, s_len=None):
                s_len = S_CHUNK_ if s_len is None else s_len
                s0 = sc * S_CHUNK_ + s_off
                orows = min(P, O_SHARD_ - ot * P)
                out_t = out_pool.tile([P, S_CHUNK_], mybir.dt.float32)
                nc.vector.tensor_scalar(
                    out=out_t[:orows, :s_len],
                    in0=psum_t[:orows, :s_len],
                    scalar1=scale_t[:orows, ot:ot + 1],
                    scalar2=bias_t[:orows, ot:ot + 1],
                    op0=mybir.AluOpType.mult,
                    op1=mybir.AluOpType.add,
                )
                nc.sync.dma_start(
                    yt[ot * P:ot * P + orows, s0:s0 + s_len],
                    out_t[:orows, :s_len],
                )

            def emit_groups(sc, rhs, tail=False):
                # kt outer / o-tile inner: each x block's last reader comes
                # early in the group sweep, so next-chunk loads spread over
                # the whole chunk instead of bunching at its tail.
                for g, (g_start, g_end) in enumerate(groups):
                    last_group = tail and g == len(groups) - 1
                    if last_group:
                        # kt-inner per o-tile: each o-tile completes ~7us
                        # apart, so evictions/output DMAs overlap the
                        # remaining matmuls. The final o-tile is further
                        # split into two s-halves for the same reason.
                        for ot in range(g_start, g_end):
                            orows = min(P, O_SHARD_ - ot * P)
                            halves = ((0, S_CHUNK_),) if ot < g_end - 1 else (
                                (0, S_CHUNK_ // 2), (S_CHUNK_ // 2, S_CHUNK_ // 2))
                            for s_off, s_len in halves:
                                ps = psum_pool.tile(
                                    [P, s_len], mybir.dt.float32,
                                    name=f"psum_{sc}_{ot}_{s_off}", tag="psum",
                                )
                                for kt in range(KT):
                                    w_slice = w_slice_for(kt, g, ot - g_start, orows)
                                    nc.tensor.matmul(
                                        ps[:orows, :], w_slice,
                                        rhs(kt)[:, s_off:s_off + s_len],
                                        start=(kt == 0), stop=(kt == KT - 1),
                                    )
                                evict(sc, ot, ps, s_off, s_len)
                        continue
                    psums = {}
                    for ot in range(g_start, g_end):
                        psums[ot] = psum_pool.tile(
                            [P, S_CHUNK_], mybir.dt.float32,
                            name=f"psum_{sc}_{ot}", tag="psum",
                        )
                    for kt in range(KT):
                        for ot in range(g_start, g_end):
                            orows = min(P, O_SHARD_ - ot * P)
                            w_slice = w_slice_for(kt, g, ot - g_start, orows)
                            nc.tensor.matmul(
                                psums[ot][:orows, :], w_slice, rhs(kt),
                                start=(kt == 0), stop=(kt == KT - 1),
                            )
                    for ot in range(g_start, g_end):
                        evict(sc, ot, psums[ot])

            # Software-pipelined emission: loads for chunk sc+1 are emitted
            # before chunk sc's matmul groups, so in the per-queue FIFO
            # streams next-chunk loads sit ahead of this chunk's PSUM
            # drains.
            prev = rhs0
            for sc in range(N_CHUNKS):
                if sc + 1 < N_CHUNKS:
                    nxt = emit_x_chunk(sc + 1)
                else:
                    nxt = None
                emit_groups(sc, prev, tail=(sc == N_CHUNKS - 1))
                prev = nxt

    nc.compile()
    return nc


_NC_CACHE = None


def _get_nc():
    global _NC_CACHE
    if _NC_CACHE is None:
        _NC_CACHE = build_bass()
    return _NC_CACHE


def run(inputs, trace=False, trace_cores=None, tmpdir=None):
    x = np.asarray(inputs["x"])
    w = np.asarray(inputs["weight_int8"])
    scale = np.asarray(inputs["scale"], dtype=np.float32)
    bias = np.asarray(inputs["bias"], dtype=np.float32)

    x2d = np.ascontiguousarray(x.reshape(S, I).astype(np.float32, copy=False))
    xtr = np.ascontiguousarray(x2d.T.astype(np.float16))  # [I, S] fp16

    KT = I // P
    col_groups = []
    c = 0
    for gsz in GROUP_SIZES:
        c1 = min(c + gsz * P, O_SHARD)
        if c < c1:
            col_groups.append((c, c1))
        c = c1

    in_maps = []
    for cid in range(N_CORES):
        sl = slice(cid * O_SHARD, (cid + 1) * O_SHARD)
        wtr = w[sl, :].T.astype(np.float16)  # [I, O_SHARD] fp16 (int8 exact)
        m = {
            "xt": xtr,
            "scale": np.ascontiguousarray(scale[sl]),
            "bias": np.ascontiguousarray(bias[sl]),
        }
        for g, (c0, c1) in enumerate(col_groups):
            wid = c1 - c0
            # wg[p, kt*wid + o] = wtr[kt*128 + p, c0 + o]
            m[f"wg{g}"] = np.ascontiguousarray(
                wtr[:, c0:c1].reshape(KT, P, wid).transpose(1, 0, 2)
                .reshape(P, KT * wid))
        in_maps.append(m)

    nc = _get_nc()
    kwargs = {}
    if trace:
        kwargs["trace"] = True
        if trace_cores is not None:
            kwargs["trace_cores"] = trace_cores
        if tmpdir is not None:
            kwargs["tmpdir"] = tmpdir
    res = run_bass_kernel_spmd(nc, in_maps, core_ids=list(range(N_CORES)), **kwargs)

    yt_full = np.concatenate([res.results[c]["yt"] for c in range(N_CORES)], axis=0)
    out = np.ascontiguousarray(yt_full.T).reshape(B, S, O).astype(np.float32, copy=False)
    if trace:
        return out, res
    return out


def kernel(**inputs) -> np.ndarray:
    return run(inputs, trace=False)


# revision 18
# speedup vs baseline: 1.0402x; 1.0140x over previous
"""CompressedLinear (int8 weight, per-row scale) on 8 Trainium2 NeuronCores.

Math: y[b,s,o] = sum_i x[b,s,i] * (w_int8[o,i] * scale[o]) + bias[o]

Strategy (tensor-parallel over out_features, per sharding hint):
  - Shard W/scale/bias rows across 8 cores (1376 rows each); x replicated.
  - Scale is applied to the matmul OUTPUT (algebraically identical), so the
    device matmuls run on the raw int8 weights in fp16 (int8 is exact in
    fp16). A single fp16 pass with host-side fp16 x bounds the output
    relative error at ~2e-4.
  - Each core computes yT[o_shard, s] = W_shard @ x^T.
  - Load scheduling is latency-driven: a dma_start costs ~0.7-1us of queue
    issue time, in-flight descriptors round-robin across the DMA engines
    (a flood starves whatever is urgent), and the gpsimd SWDGE *casting*
    path has ~6us fixed latency per block. So both operand streams are
    host-preprocessed fp16 ridden on exactly two hardware-DGE rings:
      * x blocks on the sync ring (multi-k-slice 3D-pattern blocks,
        1-k-slice head), one continuous depth-2 completion-chain across
        all chunks;
      * all weights, pre-cast and pre-blocked per PSUM o-group on the
        host, on the scalar ring the same way (the PE consumes one
        (x, w) k-slice pair per ~0.86us in a group sweep = ~296 GB/s
        joint, within two rings' bandwidth).
    The first (x, w) pair lands ~2.5us after the ~6.5us queue preamble;
    warm-up matmuls bridge the gap so the HAM clock gate is open when the
    real stream starts.
  - Per-partition affine (scale, bias) is fused into the PSUM eviction.
  - The very last PSUM group runs kt-inner per o-tile, and the final
    o-tile is split into two s-halves, so the last evictions/output DMAs
    stagger into the matmul stream instead of serializing at the end.
"""

import os
import numpy as np

import concourse.bass as bass
import concourse.tile as tile
from concourse import bacc, mybir
from concourse.bass_utils import run_bass_kernel_spmd

B = 1
S = 2048
I = 4096
O = 11008
N_CORES = 8
O_SHARD = O // N_CORES  # 1376
S_CHUNK = 512
P = 128
KB = 8    # k-slices per x block, chunks 1+
HEAD_BLOCKS = [1, 1, 2, 4, 4, 4, 4, 4, 4, 4]  # k-slices per head-stream block
GROUP_SIZES = (4, 4, 3)  # o-tiles per PSUM group


def build_bass(I_=I, O_SHARD_=O_SHARD, S_=S, S_CHUNK_=S_CHUNK):
    KT = I_ // P
    N_CHUNKS = S_ // S_CHUNK_
    OT = (O_SHARD_ + P - 1) // P

    MM_DT = mybir.dt.float16
    nc = bacc.Bacc("TRN2", target_bir_lowering=False, debug=False)

    # PSUM bank groups: 4+4+3 o-tiles so two adjacent groups fit in the
    # 8 banks and group transitions never wait on drains.
    groups = []
    g0 = 0
    for gsz in GROUP_SIZES:
        if g0 < OT:
            groups.append((g0, min(g0 + gsz, OT)))
            g0 += gsz
    g_width = [min(ge * P, O_SHARD_) - gs * P for gs, ge in groups]

    xt = nc.dram_tensor("xt", [I_, S_], mybir.dt.float16, kind="ExternalInput").ap()
    # weights pre-cast to fp16 and pre-blocked on the host, one dram
    # tensor per PSUM o-group: wg[g][p, kt*wid + o] = W^T[kt*128+p, c0+o]
    wg = [
        nc.dram_tensor(f"wg{g}", [P, KT * g_width[g]], mybir.dt.float16,
                       kind="ExternalInput").ap()
        for g in range(len(groups))
    ]
    scale = nc.dram_tensor("scale", [O_SHARD_], mybir.dt.float32, kind="ExternalInput").ap()
    bias = nc.dram_tensor("bias", [O_SHARD_], mybir.dt.float32, kind="ExternalInput").ap()
    yt = nc.dram_tensor("yt", [O_SHARD_, S_], mybir.dt.float32, kind="ExternalOutput").ap()

    full_t = O_SHARD_ // P
    rem = O_SHARD_ - full_t * P

    with tile.TileContext(nc) as tc:
        with (
            tc.tile_pool(name="wres", bufs=1) as wres_pool,
            tc.tile_pool(name="consts", bufs=1) as const_pool,
            tc.tile_pool(name="xc0", bufs=1) as x0_pool,
            tc.tile_pool(name="xcn", bufs=2 * (KT // KB)) as xn_pool,
            tc.tile_pool(name="outp", bufs=4) as out_pool,
            tc.tile_pool(name="psum", bufs=8, space="PSUM") as psum_pool,
        ):
            # PE warm-up: dependency-free matmuls on a zeroed tile keep the
            # PE busy from right after the preamble, so the HAM clock gate
            # opens (K=8/8) around when the first real matmuls flow.
            warm_sb = const_pool.tile([P, P], MM_DT)
            nc.any.memset(warm_sb[:], 0.0)
            warm_ps = psum_pool.tile([P, P], mybir.dt.float32, name="warm_ps", tag="psum")
            N_WARM = 16
            for i in range(N_WARM):
                nc.tensor.matmul(
                    warm_ps[:], warm_sb[:], warm_sb[:],
                    start=(i == 0), stop=(i == N_WARM - 1),
                )

            def chain(dd, dds, reason, head_depth=2, depth=4, head_n=4):
                # depth-2 for the first head_n blocks (low first-block
                # latency), then depth-4 (more in flight = bigger share of
                # the DMA engine pool = higher stream rate).
                d = head_depth if len(dds) < head_n else depth
                if len(dds) >= d:
                    bass._add_dep_helper(
                        dd.ins, dds[-d].ins, sync=True, reason=reason)
                dds.append(dd)

            # All x blocks ride the sync ring in ONE continuous depth-2
            # completion-chain: at most two descriptors in flight, so the
            # urgent chunk-0 head stays low-latency and later bulk chunks
            # never dilute the weight stream's bandwidth share.
            x_dds = []

            def emit_x_chunk0():
                blocks = []  # (kt0, kb, tile)
                kt0 = 0
                for i, kb in enumerate(HEAD_BLOCKS):
                    bt = x0_pool.tile([P, kb * S_CHUNK_], MM_DT, tag=f"x0b{i}")
                    src = xt[kt0 * P:(kt0 + kb) * P, 0:S_CHUNK_]
                    dd = nc.sync.dma_start(
                        bt[:].rearrange("p (kt s) -> p kt s", s=S_CHUNK_),
                        src.rearrange("(kt p) s -> p kt s", p=P))
                    chain(dd, x_dds, "x stream: low-latency head, then depth-4")
                    blocks.append((kt0, kb, bt))
                    kt0 += kb
                def rhs(kt, blocks=blocks):
                    for kt0, kb, bt in blocks:
                        if kt0 <= kt < kt0 + kb:
                            return bt[:, (kt - kt0) * S_CHUNK_:(kt - kt0 + 1) * S_CHUNK_]
                    raise KeyError(kt)
                return rhs

            def emit_x_chunk(sc):
                s0 = sc * S_CHUNK_
                blocks = []
                for b in range(KT // KB):
                    bt = xn_pool.tile([P, KB * S_CHUNK_], MM_DT, tag=f"xb{KB}")
                    src = xt[b * KB * P:(b + 1) * KB * P, s0:s0 + S_CHUNK_]
                    dd = nc.sync.dma_start(
                        bt[:].rearrange("p (kt s) -> p kt s", s=S_CHUNK_),
                        src.rearrange("(kt p) s -> p kt s", p=P))
                    chain(dd, x_dds, "x stream: bulk paced behind head")
                    blocks.append(bt)
                def rhs(kt, blocks=blocks):
                    return blocks[kt // KB][:, (kt % KB) * S_CHUNK_:(kt % KB + 1) * S_CHUNK_]
                return rhs

            rhs0 = emit_x_chunk0()

            # Weights: host-pre-cast fp16 on the scalar ring, group 0
            # first with the same small-head chaining, then groups 1/2
            # (their deadlines are one/two group sweeps out).
            w_dds = []
            w_blocks = [None] * len(groups)
            for g in range(len(groups)):
                wid = g_width[g]
                blocks = []  # (kt0, kb, tile)
                kt0 = 0
                sizes = HEAD_BLOCKS if g == 0 else [4] * (KT // 4)
                for i, kb in enumerate(sizes):
                    w_b = wres_pool.tile([P, kb * wid], MM_DT, tag=f"wg{g}_{i}")
                    dd = nc.scalar.dma_start(
                        w_b[:], wg[g][:, kt0 * wid:(kt0 + kb) * wid])
                    chain(dd, w_dds, "w stream: low-latency head, then depth-4", head_n=2)
                    blocks.append((kt0, kb, w_b))
                    kt0 += kb
                w_blocks[g] = blocks

            def w_slice_for(kt, g, ot_local, orows):
                wid = g_width[g]
                for kt0, kb, w_b in w_blocks[g]:
                    if kt0 <= kt < kt0 + kb:
                        base = (kt - kt0) * wid + ot_local * P
                        return w_b[:, base:base + orows]
                raise KeyError(kt)

            # per-partition scale/bias columns, scalar ring after the
            # weights (tiny; first needed at the first eviction ~35us in).
            scale_t = const_pool.tile([P, OT], mybir.dt.float32)
            bias_t = const_pool.tile([P, OT], mybir.dt.float32)
            if full_t:
                nc.gpsimd.dma_start(
                    scale_t[:, :full_t], scale[: full_t * P].rearrange("(t p) -> p t", p=P)
                )
                nc.gpsimd.dma_start(
                    bias_t[:, :full_t], bias[: full_t * P].rearrange("(t p) -> p t", p=P)
                )
            if rem:
                nc.gpsimd.dma_start(
                    scale_t[:rem, full_t:], scale[full_t * P:].rearrange("(t p) -> p t", p=rem)
                )
                nc.gpsimd.dma_start(
                    bias_t[:rem, full_t:], bias[full_t * P:].rearrange("(t p) -> p t", p=rem)
                )

            def evict(sc, ot, psum_t, s_off=0, s_len=None):
                s_len = S_CHUNK_ if s_len is None else s_len
                s0 = sc * S_CHUNK_ + s_off
                orows = min(P, O_SHARD_ - ot * P)
                out_t = out_pool.tile([P, S_CHUNK_], mybir.dt.float32)
                nc.vector.tensor_scalar(
                    out=out_t[:orows, :s_len],
                    in0=psum_t[:orows, :s_len],
                    scalar1=scale_t[:orows, ot:ot + 1],
                    scalar2=bias_t[:orows, ot:ot + 1],
                    op0=mybir.AluOpType.mult,
                    op1=mybir.AluOpType.add,
                )
                nc.sync.dma_start(
                    yt[ot * P:ot * P + orows, s0:s0 + s_len],
                    out_t[:orows, :s_len],
                )

            def emit_groups(sc, rhs, tail=False):
                # kt outer / o-tile inner: each x block's last reader comes
                # early in the group sweep, so next-chunk loads spread over
                # the whole chunk instead of bunching at its tail.
                for g, (g_start, g_end) in enumerate(groups):
                    last_group = tail and g == len(groups) - 1
                    if last_group:
                        # kt-inner per o-tile: each o-tile completes ~7us
                        # apart, so evictions/output DMAs overlap the
                        # remaining matmuls. The final o-tile is further
                        # split into two s-halves for the same reason.
                        for ot in range(g_start, g_end):
                            orows = min(P, O_SHARD_ - ot * P)
                            halves = ((0, S_CHUNK_),) if ot < g_end - 1 else (
                                (0, S_CHUNK_ // 2), (S_CHUNK_ // 2, S_CHUNK_ // 2))
                            for s_off, s_len in halves:
                                ps = psum_pool.tile(
                                    [P, s_len], mybir.dt.float32,
                                    name=f"psum_{sc}_{ot}_{s_off}", tag="psum",
                                )
                                for kt in range(KT):
                                    w_slice = w_slice_for(kt, g, ot - g_start, orows)
                                    nc.tensor.matmul(
                                        ps[:orows, :], w_slice,
                                        rhs(kt)[:, s_off:s_off + s_len],
                                        start=(kt == 0), stop=(kt == KT - 1),
                                    )
                                evict(sc, ot, ps, s_off, s_len)
                        continue
                    psums = {}
                    for ot in range(g_start, g_end):
                        psums[ot] = psum_pool.tile(
                            [P, S_CHUNK_], mybir.dt.float32,
                            name=f"psum_{sc}_{ot}", tag="psum",
                        )
                    for kt in range(KT):
                        for ot in range(g_start, g_end):
                            orows = min(P, O_SHARD_ - ot * P)
                            w_slice = w_slice_for(kt, g, ot - g_start, orows)
                            nc.tensor.matmul(
                                psums[ot][:orows, :], w_slice, rhs(kt),
                                start=(kt == 0), stop=(kt == KT - 1),
                            )
                    for ot in range(g_start, g_end):
                        evict(sc, ot, psums[ot])

            # Software-pipelined emission: loads for chunk sc+1 are emitted
            # before chunk sc's matmul groups, so in the per-queue FIFO
            # streams next-chunk loads sit ahead of this chunk's PSUM
            # drains.
            prev = rhs0
            for sc in range(N_CHUNKS):
                if sc + 1 < N_CHUNKS:
                    nxt = emit_x_chunk(sc + 1)
                else:
                    nxt = None
                emit_groups(sc, prev, tail=(sc == N_CHUNKS - 1))
                prev = nxt

    nc.compile()
    return nc


_NC_CACHE = None


def _get_nc():
    global _NC_CACHE
    if _NC_CACHE is None:
        _NC_CACHE = build_bass()
    return _NC_CACHE


def run(inputs, trace=False, trace_cores=None, tmpdir=None):
    x = np.asarray(inputs["x"])
    w = np.asarray(inputs["weight_int8"])
    scale = np.asarray(inputs["scale"], dtype=np.float32)
    bias = np.asarray(inputs["bias"], dtype=np.float32)

    x2d = np.ascontiguousarray(x.reshape(S, I).astype(np.float32, copy=False))
    xtr = np.ascontiguousarray(x2d.T.astype(np.float16))  # [I, S] fp16

    KT = I // P
    col_groups = []
    c = 0
    for gsz in GROUP_SIZES:
        c1 = min(c + gsz * P, O_SHARD)
        if c < c1:
            col_groups.append((c, c1))
        c = c1

    in_maps = []
    for cid in range(N_CORES):
        sl = slice(cid * O_SHARD, (cid + 1) * O_SHARD)
        wtr = w[sl, :].T.astype(np.float16)  # [I, O_SHARD] fp16 (int8 exact)
        m = {
            "xt": xtr,
            "scale": np.ascontiguousarray(scale[sl]),
            "bias": np.ascontiguousarray(bias[sl]),
        }
        for g, (c0, c1) in enumerate(col_groups):
            wid = c1 - c0
            # wg[p, kt*wid + o] = wtr[kt*128 + p, c0 + o]
            m[f"wg{g}"] = np.ascontiguousarray(
                wtr[:, c0:c1].reshape(KT, P, wid).transpose(1, 0, 2)
                .reshape(P, KT * wid))
        in_maps.append(m)

    nc = _get_nc()
    kwargs = {}
    if trace:
        kwargs["trace"] = True
        if trace_cores is not None:
            kwargs["trace_cores"] = trace_cores
        if tmpdir is not None:
            kwargs["tmpdir"] = tmpdir
    res = run_bass_kernel_spmd(nc, in_maps, core_ids=list(range(N_CORES)), **kwargs)

    yt_full = np.concatenate([res.results[c]["yt"] for c in range(N_CORES)], axis=0)
    out = np.ascontiguousarray(yt_full.T).reshape(B, S, O).astype(np.float32, copy=False)
    if trace:
        return out, res
    return out


def kernel(**inputs) -> np.ndarray:
    return run(inputs, trace=False)
